# revision 73
# baseline (speedup 1.0000x reference)
"""GCN (4x SAGEConv mean-agg + PReLU + BatchNorm, graph mean-pool) on 8 TRN2 NeuronCores.

Contract: kernel(**inputs) takes FULL inputs (as produced by setup_inputs) and
returns the FULL [G, 4H] output. Self-contained: all shapes/sharding hardcoded.

Sharding: nodes (and their in-edges, bucketed by dst) are partitioned
contiguously across 8 cores; weights replicated. The exchanged/gathered
per-node state is z = the pre-BatchNorm PReLU output, in bf16, replicated per
layer via AllGather. The BN affine h = z*s + t (from batch stats) is folded
into the NEXT layer's weights/bias on device, so the stats AllReduce and
normalization sit OFF the critical path (overlapped with the next layer's
gather stream). Graph pooling accumulates raw z per block inside the main
pass; the affine lands on the [H, G] pooled tile (t added on core 0 only)
before the final AllReduce.

Aggregation is per 128-node dst block: the block's dst-sorted edges are
chunked into 128-edge gather tiles (one SWDGE indirect DMA each, 128 x 256B
descriptors from bf16 z_full); each tile's [128, 128] bf16 indicator
(edge position -> dst offset in block, scaled 1/deg) accumulates into one
fp32 PSUM tile whose 128 rows ARE the block's nodes — no slot buffer, no
DRAM roundtrip, no inverse map. Layer 1 needs no gathers at all: agg0 is a
count-matrix matmul against the 257-row embedding table.

The device-time floor on this hardware is the Pool engine's SWDGE cost
(~1.27us per indirect DMA instruction, 128 descriptors max, independent of
descriptor size 256B..1KB); the batched GPSIMD dma_gather/dma_scatter_add
ucode that would lift it is absent from this (bedrock) image, and multi-column
offset APs are not honored by the walrus unroll (only offset column 0 is
consumed). ~1659 gather tiles/layer/core x 3 gathered layers ~= 6.3ms of
pool-queue time is the dominant cost.
"""

import ml_dtypes
import numpy as np

import concourse.bass as bass
import concourse.tile as tile
from concourse import bacc, mybir
from concourse.masks import make_identity

FP = mybir.dt.float32
BF = mybir.dt.bfloat16
I16 = mybir.dt.int16
I32 = mybir.dt.int32

N_CORES = 8
P = 128          # partitions
J = 32           # dst slots (runs) per edge-tile
TPP = 3          # edge-tiles per PSUM tile (matmul out base partition 0/32/64)
PPC = 8          # PSUM tiles per call
TPC = TPP * PPC  # 24 edge-tiles per gather/scatter call
NIDX = TPC * P   # 3072 gather indices per call
NSLOT = PPC * J  # 256 slot rows per call (32 shared per PSUM group)
L = 4
EPS = 1e-5

# SEG = dst rows per agg range (per core, multiple of 128). Each range gets
# its own agg tensor so Tile can overlap main-pass blocks of completed ranges
# with the remaining ranges' gathers/scatters.
CFG_FULL = dict(N=100_000, G=128, H=128, NV=257, SEG=3_200)


def _mkcfg(N, G, H, NV, SEG):
    assert N % N_CORES == 0
    npc = N // N_CORES
    nblk = (npc + P - 1) // P
    last = npc - (nblk - 1) * P
    assert SEG % P == 0
    return dict(
        N=N, G=G, H=H, NV=NV, SEG=SEG, NPC=npc, NBLK=nblk, LAST=last,
        NSEG=(npc + SEG - 1) // SEG,
        NVC=(NV + P - 1) // P,
        AGG_ROWS=nblk * P,
    )


# ---------------------------------------------------------------------------
# host-side preprocessing
# ---------------------------------------------------------------------------

def _prep_core(cfg, cc, src, dst, in_feat, invdeg):
    """Aggregation is per 128-node block: the block's edges are chunked into
    128-edge gather tiles; each tile's [128, 128] indicator (position -> node
    offset within block, scaled by 1/deg) accumulates into one PSUM tile
    whose 128 slot rows ARE the block's nodes in order — no slot buffer, no
    inverse map.

    Nodes are PERMUTED within the shard to balance block degree-sums: the
    top-degree nodes fill the short last block (soaking up excess edges) and
    the rest are snake-distributed by degree over the full blocks, so nearly
    every full block needs exactly ceil(mean_deg*128/128) tiles on every
    core (tb is a max across cores)."""
    npc, nblk, last = cfg["NPC"], cfg["NBLK"], cfg["LAST"]
    lo = cc * npc
    sel = (dst >= lo) & (dst < lo + npc)
    e_src = src[sel].astype(np.int64)
    e_dstl = (dst[sel] - lo).astype(np.int64)

    deg_own = np.bincount(e_dstl, minlength=npc)
    by_deg = np.argsort(-deg_own, kind="stable")   # nodes, degree descending
    node_at = np.empty(npc, np.int64)              # position -> node
    nfull = nblk - 1
    # last block: the `last` highest-degree nodes
    node_at[nfull * P:] = by_deg[:last]
    # full blocks: snake-distribute the rest by degree
    rest = by_deg[last:]                           # nfull*P nodes
    snake = np.arange(nfull * P) // nfull          # round index per node
    col = np.arange(nfull * P) % nfull
    binidx = np.where(snake % 2 == 0, col, nfull - 1 - col)
    order_in_bin = snake
    pos = binidx * P + order_in_bin
    node_at[pos] = rest
    perm_pos = np.empty(npc, np.int64)             # node -> position
    perm_pos[node_at] = np.arange(npc)

    p_dst = perm_pos[e_dstl]
    order = np.argsort(p_dst, kind="stable")
    e_src = e_src[order]
    e_dstl = e_dstl[order]
    p_dst = p_dst[order]
    eblk = p_dst // P
    counts = np.bincount(eblk, minlength=nblk)
    tb = (counts + P - 1) // P   # gather tiles needed per node block
    return dict(
        e_src=e_src, e_dstl=e_dstl, p_dst=p_dst, eblk=eblk, counts=counts,
        tb=tb, node_at=node_at, perm_pos=perm_pos,
        in_feat=in_feat, invdeg=invdeg, lo=lo,
    )


def _finish_core(cfg, core, tb, row_of_node):
    npc, nblk = cfg["NPC"], cfg["NBLK"]
    NVC = cfg["NVC"]
    invdeg = core["invdeg"]
    toff = np.concatenate([[0], np.cumsum(tb)])
    T_total = int(toff[-1])

    src_idx = np.zeros((T_total, P), np.int64)
    ind = np.zeros((T_total, P, P), np.float32)

    counts = core["counts"]
    blk_start = np.concatenate([[0], np.cumsum(counts)])[:-1]
    pos_in_blk = np.arange(len(core["e_src"])) - blk_start[core["eblk"]]
    t_e = toff[core["eblk"]] + pos_in_blk // P
    pos_e = pos_in_blk % P
    src_idx[t_e, pos_e] = row_of_node[core["e_src"]]
    ind[t_e, pos_e, core["p_dst"] % P] = invdeg[core["lo"] + core["e_dstl"]]

    src32 = np.ascontiguousarray(src_idx.T.astype(np.int32))  # [128, T_total]
    ind_dev = np.ascontiguousarray(
        ind.transpose(1, 0, 2).reshape(P, T_total * P)
        .astype(ml_dtypes.bfloat16))

    # layer-1 count matrix (1/deg folded), transposed chunks, rows in
    # permuted position order: cnt_dev[b][v%128][(v//128)*P + q] = count
    NV = cfg["NV"]
    v_e = core["in_feat"][core["e_src"]]
    cntm = np.zeros(npc * NVC * P, np.float32)
    np.add.at(cntm, core["p_dst"] * (NVC * P) + v_e, 1.0)
    invdeg_perm = invdeg[core["lo"] + core["node_at"]].astype(np.float32)
    cntm = cntm.reshape(npc, NVC * P) * invdeg_perm[:, None]
    cnt_pad = np.zeros((nblk * P, NVC * P), np.float32)
    cnt_pad[:npc] = cntm
    cb = cnt_pad.reshape(nblk, P, NVC, P)
    cnt_dev = np.ascontiguousarray(
        cb.transpose(0, 3, 2, 1).reshape(nblk, P, NVC * P)
        .astype(ml_dtypes.bfloat16))

    # layer-1 self one-hot, same transposed-chunk layout as cnt
    feat_perm = np.zeros(nblk * P, np.int64)
    feat_perm[:npc] = core["in_feat"][core["lo"] + core["node_at"]]
    eoh = np.zeros((nblk, P, NVC * P), np.float32)
    bq = np.arange(nblk * P)
    eoh[bq // P, feat_perm % P, (feat_perm // P) * P + bq % P] = \
        np.where(bq < npc, 1.0, 0.0)
    eoh_dev = np.ascontiguousarray(eoh.astype(ml_dtypes.bfloat16))

    return dict(src32=src32, ind=ind_dev, cnt=cnt_dev, eoh=eoh_dev)


def _prep(cfg, in_feat, src, dst, graph_ids, emb, W_self, W_neigh, b,
          gamma, beta, prelu_w):
    N, G, H = cfg["N"], cfg["G"], cfg["H"]
    npc, nblk = cfg["NPC"], cfg["NBLK"]
    NV, NVC, NSEG = cfg["NV"], cfg["NVC"], cfg["NSEG"]
    in_feat = np.asarray(in_feat).astype(np.int64)
    src = np.asarray(src).astype(np.int64)
    dst = np.asarray(dst).astype(np.int64)
    graph_ids = np.asarray(graph_ids).astype(np.int64)

    deg = np.bincount(dst, minlength=N)
    invdeg = (1.0 / np.clip(deg, 1, None)).astype(np.float64)

    cores = [_prep_core(cfg, cc, src, dst, in_feat, invdeg)
             for cc in range(N_CORES)]
    tb = np.maximum.reduce([c["tb"] for c in cores])  # SPMD: pad to max

    # global node -> z_full row. Per-core block-balancing permutation,
    # composed with the split-AllGather layout: z_full is the concatenation
    # of three AllGathers (half, quarter, quarter of each shard), so the
    # early exchanges fire mid-stream and only a quarter sits on the
    # layer boundary.
    hs, qs = npc // 2, npc // 4
    bounds = [0, hs, hs + qs, npc]
    row_of_node = np.empty(N, np.int64)
    for cc, c in enumerate(cores):
        p = c["perm_pos"]
        row = np.empty(npc, np.int64)
        for r in range(3):
            s, e = bounds[r], bounds[r + 1]
            m = (p >= s) & (p < e)
            row[m] = s * N_CORES + cc * (e - s) + (p[m] - s)
        row_of_node[c["lo"]:c["lo"] + npc] = row

    fins = [_finish_core(cfg, c, tb, row_of_node) for c in cores]

    cnt_g = np.clip(np.bincount(graph_ids, minlength=G), 1, None)
    emb_pad = np.zeros((NVC * P, H), np.float32)
    emb_pad[:NV] = np.asarray(emb, np.float32)
    # layer-0 weights folded into the embedding table on the host
    ew = (emb_pad @ np.asarray(W_self, np.float32)[0]).astype(ml_dtypes.bfloat16)
    cw = (emb_pad @ np.asarray(W_neigh, np.float32)[0]).astype(ml_dtypes.bfloat16)

    seg_bounds = []
    in_maps = []
    for cc, fin in enumerate(fins):
        lo = cc * npc
        gown = np.zeros(nblk * P, np.int64)
        gown[:npc] = graph_ids[lo + cores[cc]["node_at"]]
        gind = np.zeros((nblk * P, G), np.float32)
        gind[np.arange(npc), gown[:npc]] = 1.0 / cnt_g[gown[:npc]]
        gind = np.ascontiguousarray(gind.reshape(nblk, P, G))

        tflag = np.full((P, 1), 1.0 if cc == 0 else 0.0, np.float32)
        in_maps.append(dict(
            src32=fin["src32"], ind=fin["ind"],
            cnt=fin["cnt"], eoh=fin["eoh"], ew=ew, cw=cw,
            gind=gind,
            W_self=np.ascontiguousarray(np.asarray(W_self, np.float32)),
            W_neigh=np.ascontiguousarray(np.asarray(W_neigh, np.float32)),
            W_sum=np.ascontiguousarray(
                np.asarray(W_self, np.float32) + np.asarray(W_neigh, np.float32)),
            tflag=tflag,
            b_cols=np.ascontiguousarray(np.asarray(b, np.float32).T),
            gam_cols=np.ascontiguousarray(np.asarray(gamma, np.float32).T),
            bet_cols=np.ascontiguousarray(np.asarray(beta, np.float32).T),
            alp_cols=np.ascontiguousarray(np.asarray(prelu_w, np.float32).T),
        ))
    return in_maps, tuple(int(x) for x in tb)


# ---------------------------------------------------------------------------
# device program
# ---------------------------------------------------------------------------

def build_program(cfg, tb, ablate=()):
    """BN-folded pipeline: the exchanged/gathered per-node state is z = the
    pre-BatchNorm PReLU output. h = z*s + t (per-channel affine from batch
    stats) is folded into the next layer's weights on device:
      rst_pre[l+1] = z @ (diag(s_l) W_self) + agg(z) @ (diag(s_l) W_neigh)
                     + (t_l @ (W_self+W_neigh) + b)          [min in-deg >= 1]
    so the BN stats AllReduce and the affine are OFF the critical path
    (computed while the next layer's gather stream runs), and the old pass B
    (normalize + transpose + pool) is fused into pass A. Graph pooling
    accumulates raw z; the affine is applied to the [H, G] pooled tile
    (t added on core 0 only) before the final AllReduce."""
    N, G, H = cfg["N"], cfg["G"], cfg["H"]
    npc, nblk, last = cfg["NPC"], cfg["NBLK"], cfg["LAST"]
    NVC = cfg["NVC"]
    agg_rows = cfg["AGG_ROWS"]
    toff = [0]
    for t in tb:
        toff.append(toff[-1] + t)
    T_total = toff[-1]
    TBMAX = max(tb)

    nc = bacc.Bacc("TRN2", target_bir_lowering=False, debug=False,
                   num_devices=N_CORES)

    src32_d = nc.declare_dram_parameter("src32", [P, T_total], I32,
                                        isOutput=False)
    ind_d = nc.declare_dram_parameter("ind", [P, T_total * P], BF, isOutput=False)
    cnt_d = nc.declare_dram_parameter("cnt", [nblk, P, NVC * P], BF, isOutput=False)
    eoh_d = nc.declare_dram_parameter("eoh", [nblk, P, NVC * P], BF,
                                      isOutput=False)
    ew_d = nc.declare_dram_parameter("ew", [NVC * P, H], BF, isOutput=False)
    cw_d = nc.declare_dram_parameter("cw", [NVC * P, H], BF, isOutput=False)
    gind_d = nc.declare_dram_parameter("gind", [nblk, P, G], FP, isOutput=False)
    ws_d = nc.declare_dram_parameter("W_self", [L, H, H], FP, isOutput=False)
    wn_d = nc.declare_dram_parameter("W_neigh", [L, H, H], FP, isOutput=False)
    wsum_d = nc.declare_dram_parameter("W_sum", [L, H, H], FP, isOutput=False)
    tflag_d = nc.declare_dram_parameter("tflag", [P, 1], FP, isOutput=False)
    bcol_d = nc.declare_dram_parameter("b_cols", [H, L], FP, isOutput=False)
    gcol_d = nc.declare_dram_parameter("gam_cols", [H, L], FP, isOutput=False)
    becol_d = nc.declare_dram_parameter("bet_cols", [H, L], FP, isOutput=False)
    acol_d = nc.declare_dram_parameter("alp_cols", [H, L], FP, isOutput=False)
    out_d = nc.declare_dram_parameter("out", [G, L * H], FP, isOutput=True)

    # z exchanged/gathered in bf16: halves the AllGather on the critical path
    # and the per-edge gather bytes; everything downstream accumulates fp32
    h_shard = nc.dram_tensor("h_shard", [npc, H], BF)
    # double-buffered: the mid-stream half-AllGather of layer l's z must not
    # clobber the table layer l's own remaining gathers still read
    h_fulls = [nc.dram_tensor(f"h_full{i}", [N, H], BF, addr_space="Shared")
               for i in range(2)]
    stats_loc = nc.dram_tensor("stats_loc", [2, H], FP)
    stats_red = nc.dram_tensor("stats_red", [2, H], FP, addr_space="Shared")
    # pooled z held TRANSPOSED [H, G] so the channel affine uses per-partition
    # scalars; transposed back to [G, H] only at the very end
    pool_loc = nc.dram_tensor("pool_loc", [L, H, G], FP)
    pool_red = nc.dram_tensor("pool_red", [L, H, G], FP, addr_space="Shared")

    groups = [list(range(N_CORES))]

    with tile.TileContext(nc) as tc:
        with (
            tc.tile_pool(name="res", bufs=1) as res,
            tc.tile_pool(name="wrk", bufs=3) as wrk,
            tc.tile_pool(name="gat", bufs=3) as gat,
            tc.tile_pool(name="ps_slot", bufs=3, space="PSUM") as ps_slot,
            tc.tile_pool(name="ps_tp", bufs=2, space="PSUM") as ps_tp,
            tc.tile_pool(name="ps_rst", bufs=2, space="PSUM") as ps_rst,
            tc.tile_pool(name="ps_pool", bufs=1, space="PSUM") as ps_pool,
        ):
            ident = res.tile([P, P], FP, tag="ident")
            make_identity(nc, ident[:])

            src32_sb = res.tile([P, T_total], I32, tag="src32")
            nc.sync.dma_start(src32_sb[:], src32_d[:])
            ew_sb = res.tile([P, NVC * H], BF, tag="ew")
            cw_sb = res.tile([P, NVC * H], BF, tag="cw")
            for c in range(NVC):
                nc.sync.dma_start(ew_sb[:, c * H:(c + 1) * H],
                                  ew_d[c * P:(c + 1) * P, :])
                nc.sync.dma_start(cw_sb[:, c * H:(c + 1) * H],
                                  cw_d[c * P:(c + 1) * P, :])
            ws_sb = res.tile([P, L * H], FP, tag="ws")
            wn_sb = res.tile([P, L * H], FP, tag="wn")
            wsum_sb = res.tile([P, L * H], FP, tag="wsum")
            for l in range(L):
                nc.sync.dma_start(ws_sb[:, l * H:(l + 1) * H], ws_d[l])
                nc.sync.dma_start(wn_sb[:, l * H:(l + 1) * H], wn_d[l])
                nc.sync.dma_start(wsum_sb[:, l * H:(l + 1) * H], wsum_d[l])
            tflag_sb = res.tile([P, 1], FP, tag="tflag")
            nc.sync.dma_start(tflag_sb[:], tflag_d[:])
            bcol_sb = res.tile([P, L], FP, tag="bcol")
            nc.sync.dma_start(bcol_sb[:], bcol_d[:])
            gcol_sb = res.tile([P, L], FP, tag="gcol")
            nc.sync.dma_start(gcol_sb[:], gcol_d[:])
            becol_sb = res.tile([P, L], FP, tag="becol")
            nc.sync.dma_start(becol_sb[:], becol_d[:])
            acol_sb = res.tile([P, L], FP, tag="acol")
            nc.sync.dma_start(acol_sb[:], acol_d[:])

            h_stage = res.tile([P, nblk * P], FP, tag="hstage")
            stats_sum = res.tile([P, nblk], FP, tag="ssum")
            stats_sq = res.tile([P, nblk], FP, tag="ssq")
            scratch = res.tile([P, P], FP, tag="scratch")
            eps_col = res.tile([P, 1], FP, tag="eps")
            nc.vector.memset(eps_col[:], float(EPS))
            # per-layer BN affine (s, t), folded weights and bias columns
            s_all = res.tile([P, L], FP, tag="sall")
            t_all = res.tile([P, L], FP, tag="tall")
            wsf_sb = res.tile([P, L * H], FP, tag="wsf")
            wnf_sb = res.tile([P, L * H], FP, tag="wnf")
            biasf_sb = res.tile([P, L], FP, tag="biasf")
            pl_all = res.tile([P, L * H], FP, tag="plall")

            def emit_agg_block(bI, h_full):
                """Gather + indicator-accumulate the 128-node block's
                aggregation directly in PSUM; returns node-row agg tile."""
                nt = tb[bI]
                t0 = toff[bI]
                gt = gat.tile([P, TBMAX * H], BF, tag="g")
                if "gather" not in ablate:
                    for ti in range(nt):
                        nc.gpsimd.indirect_dma_start(
                            out=gt[:, ti * H:(ti + 1) * H],
                            out_offset=None, in_=h_full[:],
                            in_offset=bass.IndirectOffsetOnAxis(
                                ap=src32_sb[:, t0 + ti:t0 + ti + 1],
                                axis=0))
                it = wrk.tile([P, TBMAX * P], BF, tag="indblk")
                nc.sync.dma_start(
                    it[:, :nt * P], ind_d[:, t0 * P:(t0 + nt) * P])
                ps = ps_slot.tile([P, H], FP, tag="slot")
                for ti in range(nt):
                    nc.tensor.matmul(
                        ps[:],
                        lhsT=it[:, ti * P:(ti + 1) * P],
                        rhs=gt[:, ti * H:(ti + 1) * H],
                        start=(ti == 0), stop=(ti == nt - 1))
                ab = wrk.tile([P, H], FP, tag="mablk")
                nc.vector.tensor_copy(ab[:], ps[:])
                return ab

            def emit_stats_post(j):
                """s_j, t_j from the (already AllReduced) stats of z^j; fold
                layer j+1's weights/bias; apply the pool affine for layer j."""
                sxr = wrk.tile([P, 1], FP, tag="sxr")
                nc.sync.dma_start(sxr[:, 0:1], stats_red[0:1, :])
                sqr = wrk.tile([P, 1], FP, tag="sqr")
                nc.sync.dma_start(sqr[:, 0:1], stats_red[1:2, :])
                mu = wrk.tile([P, 1], FP, tag="mu")
                nc.scalar.mul(mu[:], sxr[:], 1.0 / N)
                ex2 = wrk.tile([P, 1], FP, tag="ex2")
                nc.scalar.mul(ex2[:], sqr[:], 1.0 / N)
                mu2 = wrk.tile([P, 1], FP, tag="mu2")
                nc.scalar.square(mu2[:], mu[:])
                var = wrk.tile([P, 1], FP, tag="var")
                nc.vector.tensor_sub(var[:], ex2[:], mu2[:])
                sd = wrk.tile([P, 1], FP, tag="sd")
                nc.scalar.activation(sd[:], var[:],
                                     mybir.ActivationFunctionType.Sqrt,
                                     bias=eps_col[:])
                rstd = wrk.tile([P, 1], FP, tag="rstd")
                nc.vector.reciprocal(rstd[:], sd[:])
                s_col = s_all[:, j:j + 1]
                t_col = t_all[:, j:j + 1]
                nc.vector.tensor_mul(s_col, rstd[:], gcol_sb[:, j:j + 1])
                msc = wrk.tile([P, 1], FP, tag="msc")
                nc.vector.tensor_mul(msc[:], mu[:], s_col)
                nc.vector.tensor_sub(t_col, becol_sb[:, j:j + 1], msc[:])
                if j < L - 1:
                    ln = j + 1
                    nc.vector.tensor_scalar_mul(
                        wsf_sb[:, ln * H:(ln + 1) * H],
                        ws_sb[:, ln * H:(ln + 1) * H], s_col)
                    nc.vector.tensor_scalar_mul(
                        wnf_sb[:, ln * H:(ln + 1) * H],
                        wn_sb[:, ln * H:(ln + 1) * H], s_col)
                    ps_b = ps_rst.tile([P, H], FP, tag="rst")
                    nc.tensor.matmul(ps_b[:, 0:1],
                                     lhsT=wsum_sb[:, ln * H:(ln + 1) * H],
                                     rhs=t_col, start=True, stop=True)
                    nc.vector.tensor_add(biasf_sb[:, ln:ln + 1], ps_b[:, 0:1],
                                         bcol_sb[:, ln:ln + 1])
                # pool affine for layer j: [H, G] = s*poolT + t (core 0 only)
                ps_t = ps_tp.tile([P, P], FP, tag="tp")
                nc.tensor.transpose(out=ps_t[:],
                                    in_=pl_all[:, j * H:(j + 1) * H],
                                    identity=ident[:])
                poolT = wrk.tile([P, P], FP, tag="poolT")
                nc.scalar.copy(poolT[:], ps_t[:])
                tf = wrk.tile([P, 1], FP, tag="tf")
                nc.vector.tensor_mul(tf[:], t_col, tflag_sb[:])
                pla = wrk.tile([P, G], FP, tag="pla")
                nc.vector.scalar_tensor_tensor(
                    pla[:], poolT[:, :G], s_col, tf[:].to_broadcast([P, G]),
                    op0=mybir.AluOpType.mult, op1=mybir.AluOpType.add)
                nc.sync.dma_start(pool_loc[j], pla[:])

            def emit_block(l, bI, ps_p):
                    nn = last if bI == nblk - 1 else P
                    ps_r = ps_rst.tile([P, H], FP, tag="rst")
                    if l == 0:
                        # rst_pre0^T = (emb@Ws)^T EohT + (emb@Wn)^T cntT,
                        # all host-folded: 6 accumulating matmuls, no gathers
                        eoh_sb = wrk.tile([P, NVC * P], BF, tag="eohblk")
                        nc.sync.dma_start(eoh_sb[:], eoh_d[bI])
                        cnt_sb = wrk.tile([P, NVC * P], BF, tag="cntblk")
                        nc.sync.dma_start(cnt_sb[:], cnt_d[bI])
                        for cv in range(NVC):
                            nc.tensor.matmul(
                                ps_r[:],
                                lhsT=ew_sb[:, cv * H:(cv + 1) * H],
                                rhs=eoh_sb[:, cv * P:(cv + 1) * P],
                                start=(cv == 0), stop=False)
                        for cv in range(NVC):
                            nc.tensor.matmul(
                                ps_r[:],
                                lhsT=cw_sb[:, cv * H:(cv + 1) * H],
                                rhs=cnt_sb[:, cv * P:(cv + 1) * P],
                                start=False, stop=(cv == NVC - 1))
                        bc = bcol_sb[:, 0:1]
                    else:
                        ab = emit_agg_block(bI, h_fulls[(l - 1) % 2])
                        ps_t = ps_tp.tile([P, P], FP, tag="tp")
                        nc.tensor.transpose(out=ps_t[:], in_=ab[:],
                                            identity=ident[:])
                        aT = wrk.tile([P, P], FP, tag="aT")
                        nc.scalar.copy(aT[:], ps_t[:])
                        rhs_self = h_stage[:, bI * P:(bI + 1) * P]
                        bc = biasf_sb[:, l:l + 1]
                        nc.tensor.matmul(ps_r[:],
                                         lhsT=wsf_sb[:, l * H:(l + 1) * H],
                                         rhs=rhs_self, start=True, stop=False)
                        nc.tensor.matmul(ps_r[:],
                                         lhsT=wnf_sb[:, l * H:(l + 1) * H],
                                         rhs=aT[:], start=False, stop=True)

                    t1 = wrk.tile([P, P], FP, tag="t1")
                    nc.scalar.activation(t1[:], ps_r[:],
                                         mybir.ActivationFunctionType.Relu,
                                         bias=bc)
                    neg = wrk.tile([P, P], FP, tag="neg")
                    nc.vector.tensor_scalar(
                        neg[:], ps_r[:], bc, 0.0,
                        op0=mybir.AluOpType.add, op1=mybir.AluOpType.min)
                    zb = h_stage[:, bI * P:(bI + 1) * P]
                    if nn == P:
                        nc.vector.scalar_tensor_tensor(
                            zb, neg[:], acol_sb[:, l:l + 1], t1[:],
                            op0=mybir.AluOpType.mult, op1=mybir.AluOpType.add,
                            accum_out=stats_sum[:, bI:bI + 1])
                        nc.scalar.activation(scratch[:], zb,
                                             mybir.ActivationFunctionType.Square,
                                             accum_out=stats_sq[:, bI:bI + 1])
                    else:
                        nc.vector.scalar_tensor_tensor(
                            h_stage[:, bI * P:bI * P + nn],
                            neg[:, :nn], acol_sb[:, l:l + 1], t1[:, :nn],
                            op0=mybir.AluOpType.mult, op1=mybir.AluOpType.add,
                            accum_out=stats_sum[:, bI:bI + 1])
                        nc.vector.scalar_tensor_tensor(
                            h_stage[:, bI * P + nn:(bI + 1) * P],
                            neg[:, nn:], acol_sb[:, l:l + 1], t1[:, nn:],
                            op0=mybir.AluOpType.mult, op1=mybir.AluOpType.add)
                        nc.scalar.activation(
                            scratch[:, :nn], h_stage[:, bI * P:bI * P + nn],
                            mybir.ActivationFunctionType.Square,
                            accum_out=stats_sq[:, bI:bI + 1])

                    # fused tail (old pass B): transpose z to node rows,
                    # write the shard, accumulate the raw-z pool
                    ps_t2 = ps_tp.tile([P, P], FP, tag="tp")
                    nc.tensor.transpose(out=ps_t2[:], in_=zb,
                                        identity=ident[:])
                    hnm = wrk.tile([P, P], FP, tag="hnm")
                    nc.scalar.copy(hnm[:], ps_t2[:])
                    if l < L - 1:
                        hnm_bf = wrk.tile([P, P], BF, tag="hnmbf")
                        nc.scalar.copy(hnm_bf[:], ps_t2[:])
                        nc.sync.dma_start(
                            h_shard[bI * P:bI * P + nn, :], hnm_bf[:nn, :])
                    gb = wrk.tile([P, G], FP, tag="gblk")
                    nc.sync.dma_start(gb[:], gind_d[bI])
                    nc.tensor.matmul(ps_p[:G, :], lhsT=gb[:], rhs=hnm[:],
                                     start=(bI == 0), stop=(bI == nblk - 1))

            def emit_layer_tail(l, ps_p):
                nc.vector.tensor_copy(pl_all[:G, l * H:(l + 1) * H],
                                      ps_p[:G, :])
                # per-channel z sums for this layer's BN stats
                sx = wrk.tile([P, 1], FP, tag="sx")
                nc.vector.tensor_reduce(sx[:], stats_sum[:],
                                        axis=mybir.AxisListType.X,
                                        op=mybir.AluOpType.add)
                sq = wrk.tile([P, 1], FP, tag="sq")
                nc.vector.tensor_reduce(sq[:], stats_sq[:],
                                        axis=mybir.AxisListType.X,
                                        op=mybir.AluOpType.add)
                nc.sync.dma_start(stats_loc[0:1, :], sx[:, 0:1])
                nc.sync.dma_start(stats_loc[1:2, :], sq[:, 0:1])

            # ---------------- schedule ----------------
            # exchange regions (positions) and the block after which each
            # region's pass A is certainly done (region end block + margin)
            HS, QS = npc // 2, npc // 4
            AG_BOUNDS = [0, HS, HS + QS, npc]
            AG_AT_BLK = {64: 0, 80: 1}     # region 2 fires at layer end

            def emit_ag(l, r):
                h_full = h_fulls[l % 2]
                s, e = AG_BOUNDS[r], AG_BOUNDS[r + 1]
                nc.gpsimd.collective_compute(
                    "AllGather", mybir.AluOpType.bypass,
                    replica_groups=groups,
                    ins=[h_shard[s:e]],
                    outs=[h_full[s * N_CORES:e * N_CORES]])

            for l in range(L):
                ps_p = ps_pool.tile([P, H], FP, tag="pool")
                if l > 0:
                    # the previous layer's stats postprocessing overlaps this
                    # layer's gather stream (its deps resolve early; only the
                    # first W-matmuls wait on the folded weights)
                    emit_stats_post(l - 1)
                for bI in range(nblk):
                    emit_block(l, bI, ps_p)
                    if l < L - 1 and bI in AG_AT_BLK:
                        # early shard-region exchanges fire mid-stream
                        emit_ag(l, AG_AT_BLK[bI])
                emit_layer_tail(l, ps_p)
                if l < L - 1:
                    emit_ag(l, 2)
                nc.gpsimd.collective_compute(
                    "AllReduce", mybir.AluOpType.add, replica_groups=groups,
                    ins=[stats_loc[:]], outs=[stats_red[:]])
            emit_stats_post(L - 1)

            nc.gpsimd.collective_compute(
                "AllReduce", mybir.AluOpType.add, replica_groups=groups,
                ins=[pool_loc[:]], outs=[pool_red[:]])
            for l in range(L):
                pr = wrk.tile([P, G], FP, tag="pr")
                nc.sync.dma_start(pr[:], pool_red[l])
                ps_o = ps_tp.tile([P, P], FP, tag="tp")
                nc.tensor.transpose(out=ps_o[:], in_=pr[:], identity=ident[:])
                ob = wrk.tile([P, H], FP, tag="ob")
                nc.scalar.copy(ob[:], ps_o[:])
                nc.sync.dma_start(out_d[:, l * H:(l + 1) * H], ob[:G, :])

    nc.compile()
    return nc


# ---------------------------------------------------------------------------
# entry point
# ---------------------------------------------------------------------------

_CACHE = {}


def _run(cfg, inputs, trace=False):
    from concourse.bass_utils import run_bass_kernel_spmd
    in_maps, tb = _prep(cfg, **inputs)
    key = (cfg["N"], cfg["G"], cfg["H"], tb)
    if key not in _CACHE:
        _CACHE[key] = build_program(cfg, tb)
    nc = _CACHE[key]
    last_exc = None
    for attempt in range(3):
        try:
            return run_bass_kernel_spmd(nc, in_maps, list(range(N_CORES)),
                                        trace=trace)
        except Exception as e:  # rare transient device-unrecoverable errors
            last_exc = e
            try:
                import jax
                import jax.extend.backend
                jax.clear_caches()
                jax.extend.backend.clear_backends()
            except Exception:
                pass
    raise last_exc


def kernel(in_feat, src, dst, graph_ids, emb, W_self, W_neigh, b,
           gamma, beta, prelu_w):
    cfg = _mkcfg(**CFG_FULL)
    res = _run(cfg, dict(
        in_feat=in_feat, src=src, dst=dst, graph_ids=graph_ids, emb=emb,
        W_self=W_self, W_neigh=W_neigh, b=b, gamma=gamma, beta=beta,
        prelu_w=prelu_w))
    return np.asarray(res.results[0]["out"], np.float32)



# revision 74
# speedup vs baseline: 1.0398x; 1.0398x over previous
"""GCN (4x SAGEConv mean-agg + PReLU + BatchNorm, graph mean-pool) on 8 TRN2 NeuronCores.

Contract: kernel(**inputs) takes FULL inputs (as produced by setup_inputs) and
returns the FULL [G, 4H] output. Self-contained: all shapes/sharding hardcoded.

Sharding: nodes (and their in-edges, bucketed by dst) are partitioned
contiguously across 8 cores; weights replicated. The exchanged/gathered
per-node state is z = the pre-BatchNorm PReLU output, in bf16, replicated per
layer via AllGather. The BN affine h = z*s + t (from batch stats) is folded
into the NEXT layer's weights/bias on device, so the stats AllReduce and
normalization sit OFF the critical path (overlapped with the next layer's
gather stream). Graph pooling accumulates raw z per block inside the main
pass; the affine lands on the [H, G] pooled tile (t added on core 0 only)
before the final AllReduce.

Aggregation is per 128-node dst block: the block's dst-sorted edges are
chunked into 128-edge gather tiles (one SWDGE indirect DMA each, 128 x 256B
descriptors from bf16 z_full); each tile's [128, 128] bf16 indicator
(edge position -> dst offset in block, scaled 1/deg) accumulates into one
fp32 PSUM tile whose 128 rows ARE the block's nodes — no slot buffer, no
DRAM roundtrip, no inverse map. Layer 1 needs no gathers at all: agg0 is a
count-matrix matmul against the 257-row embedding table.

The device-time floor on this hardware is the Pool engine's SWDGE cost
(~1.27us per indirect DMA instruction, 128 descriptors max, independent of
descriptor size 256B..1KB); the batched GPSIMD dma_gather/dma_scatter_add
ucode that would lift it is absent from this (bedrock) image, and multi-column
offset APs are not honored by the walrus unroll (only offset column 0 is
consumed). ~1659 gather tiles/layer/core x 3 gathered layers ~= 6.3ms of
pool-queue time is the dominant cost.
"""

import ml_dtypes
import numpy as np

import concourse.bass as bass
import concourse.tile as tile
from concourse import bacc, mybir
from concourse.masks import make_identity

FP = mybir.dt.float32
BF = mybir.dt.bfloat16
I16 = mybir.dt.int16
I32 = mybir.dt.int32

N_CORES = 8
P = 128          # partitions
J = 32           # dst slots (runs) per edge-tile
TPP = 3          # edge-tiles per PSUM tile (matmul out base partition 0/32/64)
PPC = 8          # PSUM tiles per call
TPC = TPP * PPC  # 24 edge-tiles per gather/scatter call
NIDX = TPC * P   # 3072 gather indices per call
NSLOT = PPC * J  # 256 slot rows per call (32 shared per PSUM group)
L = 4
EPS = 1e-5

# SEG = dst rows per agg range (per core, multiple of 128). Each range gets
# its own agg tensor so Tile can overlap main-pass blocks of completed ranges
# with the remaining ranges' gathers/scatters.
CFG_FULL = dict(N=100_000, G=128, H=128, NV=257, SEG=3_200)


def _mkcfg(N, G, H, NV, SEG):
    assert N % N_CORES == 0
    npc = N // N_CORES
    nblk = (npc + P - 1) // P
    last = npc - (nblk - 1) * P
    assert SEG % P == 0
    return dict(
        N=N, G=G, H=H, NV=NV, SEG=SEG, NPC=npc, NBLK=nblk, LAST=last,
        NSEG=(npc + SEG - 1) // SEG,
        NVC=(NV + P - 1) // P,
        AGG_ROWS=nblk * P,
    )


# ---------------------------------------------------------------------------
# host-side preprocessing
# ---------------------------------------------------------------------------

def _prep_core(cfg, cc, src, dst, in_feat, invdeg):
    """Aggregation is per 128-node block: the block's edges are chunked into
    128-edge gather tiles; each tile's [128, 128] indicator (position -> node
    offset within block, scaled by 1/deg) accumulates into one PSUM tile
    whose 128 slot rows ARE the block's nodes in order — no slot buffer, no
    inverse map.

    Nodes are PERMUTED within the shard to balance block degree-sums: the
    top-degree nodes fill the short last block (soaking up excess edges) and
    the rest are snake-distributed by degree over the full blocks, so nearly
    every full block needs exactly ceil(mean_deg*128/128) tiles on every
    core (tb is a max across cores)."""
    npc, nblk, last = cfg["NPC"], cfg["NBLK"], cfg["LAST"]
    lo = cc * npc
    sel = (dst >= lo) & (dst < lo + npc)
    e_src = src[sel].astype(np.int64)
    e_dstl = (dst[sel] - lo).astype(np.int64)

    deg_own = np.bincount(e_dstl, minlength=npc)
    by_deg = np.argsort(-deg_own, kind="stable")   # nodes, degree descending
    node_at = np.empty(npc, np.int64)              # position -> node
    nfull = nblk - 1
    # last block: the `last` highest-degree nodes
    node_at[nfull * P:] = by_deg[:last]
    # full blocks: snake-distribute the rest by degree
    rest = by_deg[last:]                           # nfull*P nodes
    snake = np.arange(nfull * P) // nfull          # round index per node
    col = np.arange(nfull * P) % nfull
    binidx = np.where(snake % 2 == 0, col, nfull - 1 - col)
    order_in_bin = snake
    pos = binidx * P + order_in_bin
    node_at[pos] = rest
    perm_pos = np.empty(npc, np.int64)             # node -> position
    perm_pos[node_at] = np.arange(npc)

    p_dst = perm_pos[e_dstl]
    order = np.argsort(p_dst, kind="stable")
    e_src = e_src[order]
    e_dstl = e_dstl[order]
    p_dst = p_dst[order]
    eblk = p_dst // P
    counts = np.bincount(eblk, minlength=nblk)
    tb = (counts + P - 1) // P   # gather tiles needed per node block
    return dict(
        e_src=e_src, e_dstl=e_dstl, p_dst=p_dst, eblk=eblk, counts=counts,
        tb=tb, node_at=node_at, perm_pos=perm_pos,
        in_feat=in_feat, invdeg=invdeg, lo=lo,
    )


def _finish_core(cfg, core, tb, row_of_node):
    npc, nblk = cfg["NPC"], cfg["NBLK"]
    NVC = cfg["NVC"]
    invdeg = core["invdeg"]
    toff = np.concatenate([[0], np.cumsum(tb)])
    T_total = int(toff[-1])

    src_idx = np.zeros((T_total, P), np.int64)
    ind = np.zeros((T_total, P, P), np.float32)

    counts = core["counts"]
    blk_start = np.concatenate([[0], np.cumsum(counts)])[:-1]
    pos_in_blk = np.arange(len(core["e_src"])) - blk_start[core["eblk"]]
    t_e = toff[core["eblk"]] + pos_in_blk // P
    pos_e = pos_in_blk % P
    src_idx[t_e, pos_e] = row_of_node[core["e_src"]]
    ind[t_e, pos_e, core["p_dst"] % P] = invdeg[core["lo"] + core["e_dstl"]]

    src32 = np.ascontiguousarray(src_idx.T.astype(np.int32))  # [128, T_total]
    ind_dev = np.ascontiguousarray(
        ind.transpose(1, 0, 2).reshape(P, T_total * P)
        .astype(ml_dtypes.bfloat16))

    # layer-1 count matrix (1/deg folded), transposed chunks, rows in
    # permuted position order: cnt_dev[b][v%128][(v//128)*P + q] = count
    NV = cfg["NV"]
    v_e = core["in_feat"][core["e_src"]]
    cntm = np.zeros(npc * NVC * P, np.float32)
    np.add.at(cntm, core["p_dst"] * (NVC * P) + v_e, 1.0)
    invdeg_perm = invdeg[core["lo"] + core["node_at"]].astype(np.float32)
    cntm = cntm.reshape(npc, NVC * P) * invdeg_perm[:, None]
    cnt_pad = np.zeros((nblk * P, NVC * P), np.float32)
    cnt_pad[:npc] = cntm
    cb = cnt_pad.reshape(nblk, P, NVC, P)
    cnt_dev = np.ascontiguousarray(
        cb.transpose(0, 3, 2, 1).reshape(nblk, P, NVC * P)
        .astype(ml_dtypes.bfloat16))

    # layer-1 self one-hot, same transposed-chunk layout as cnt
    feat_perm = np.zeros(nblk * P, np.int64)
    feat_perm[:npc] = core["in_feat"][core["lo"] + core["node_at"]]
    eoh = np.zeros((nblk, P, NVC * P), np.float32)
    bq = np.arange(nblk * P)
    eoh[bq // P, feat_perm % P, (feat_perm // P) * P + bq % P] = \
        np.where(bq < npc, 1.0, 0.0)
    eoh_dev = np.ascontiguousarray(eoh.astype(ml_dtypes.bfloat16))

    return dict(src32=src32, ind=ind_dev, cnt=cnt_dev, eoh=eoh_dev)


def _prep(cfg, in_feat, src, dst, graph_ids, emb, W_self, W_neigh, b,
          gamma, beta, prelu_w):
    N, G, H = cfg["N"], cfg["G"], cfg["H"]
    npc, nblk = cfg["NPC"], cfg["NBLK"]
    NV, NVC, NSEG = cfg["NV"], cfg["NVC"], cfg["NSEG"]
    in_feat = np.asarray(in_feat).astype(np.int64)
    src = np.asarray(src).astype(np.int64)
    dst = np.asarray(dst).astype(np.int64)
    graph_ids = np.asarray(graph_ids).astype(np.int64)

    deg = np.bincount(dst, minlength=N)
    invdeg = (1.0 / np.clip(deg, 1, None)).astype(np.float64)

    cores = [_prep_core(cfg, cc, src, dst, in_feat, invdeg)
             for cc in range(N_CORES)]
    tb = np.maximum.reduce([c["tb"] for c in cores])  # SPMD: pad to max

    # global node -> z_full row. Per-core block-balancing permutation,
    # composed with the split-AllGather layout: z_full = [AllGather of all
    # cores' first half-shards | AllGather of second half-shards], so the
    # first exchange can fire mid-stream.
    hs = npc // 2
    row_of_node = np.empty(N, np.int64)
    for cc, c in enumerate(cores):
        p = c["perm_pos"]
        row_of_node[c["lo"]:c["lo"] + npc] = np.where(
            p < hs, cc * hs + p, N // 2 + cc * hs + (p - hs))

    fins = [_finish_core(cfg, c, tb, row_of_node) for c in cores]

    cnt_g = np.clip(np.bincount(graph_ids, minlength=G), 1, None)
    emb_pad = np.zeros((NVC * P, H), np.float32)
    emb_pad[:NV] = np.asarray(emb, np.float32)
    # layer-0 weights folded into the embedding table on the host
    ew = (emb_pad @ np.asarray(W_self, np.float32)[0]).astype(ml_dtypes.bfloat16)
    cw = (emb_pad @ np.asarray(W_neigh, np.float32)[0]).astype(ml_dtypes.bfloat16)

    seg_bounds = []
    in_maps = []
    for cc, fin in enumerate(fins):
        lo = cc * npc
        gown = np.zeros(nblk * P, np.int64)
        gown[:npc] = graph_ids[lo + cores[cc]["node_at"]]
        gind = np.zeros((nblk * P, G), np.float32)
        gind[np.arange(npc), gown[:npc]] = 1.0 / cnt_g[gown[:npc]]
        gind = np.ascontiguousarray(gind.reshape(nblk, P, G))

        tflag = np.full((P, 1), 1.0 if cc == 0 else 0.0, np.float32)
        in_maps.append(dict(
            src32=fin["src32"], ind=fin["ind"],
            cnt=fin["cnt"], eoh=fin["eoh"], ew=ew, cw=cw,
            gind=gind,
            W_self=np.ascontiguousarray(np.asarray(W_self, np.float32)),
            W_neigh=np.ascontiguousarray(np.asarray(W_neigh, np.float32)),
            W_sum=np.ascontiguousarray(
                np.asarray(W_self, np.float32) + np.asarray(W_neigh, np.float32)),
            tflag=tflag,
            b_cols=np.ascontiguousarray(np.asarray(b, np.float32).T),
            gam_cols=np.ascontiguousarray(np.asarray(gamma, np.float32).T),
            bet_cols=np.ascontiguousarray(np.asarray(beta, np.float32).T),
            alp_cols=np.ascontiguousarray(np.asarray(prelu_w, np.float32).T),
        ))
    return in_maps, tuple(int(x) for x in tb)


# ---------------------------------------------------------------------------
# device program
# ---------------------------------------------------------------------------

def build_program(cfg, tb, ablate=()):
    """BN-folded pipeline: the exchanged/gathered per-node state is z = the
    pre-BatchNorm PReLU output. h = z*s + t (per-channel affine from batch
    stats) is folded into the next layer's weights on device:
      rst_pre[l+1] = z @ (diag(s_l) W_self) + agg(z) @ (diag(s_l) W_neigh)
                     + (t_l @ (W_self+W_neigh) + b)          [min in-deg >= 1]
    so the BN stats AllReduce and the affine are OFF the critical path
    (computed while the next layer's gather stream runs), and the old pass B
    (normalize + transpose + pool) is fused into pass A. Graph pooling
    accumulates raw z; the affine is applied to the [H, G] pooled tile
    (t added on core 0 only) before the final AllReduce."""
    N, G, H = cfg["N"], cfg["G"], cfg["H"]
    npc, nblk, last = cfg["NPC"], cfg["NBLK"], cfg["LAST"]
    NVC = cfg["NVC"]
    agg_rows = cfg["AGG_ROWS"]
    toff = [0]
    for t in tb:
        toff.append(toff[-1] + t)
    T_total = toff[-1]
    TBMAX = max(tb)

    nc = bacc.Bacc("TRN2", target_bir_lowering=False, debug=False,
                   num_devices=N_CORES)

    src32_d = nc.declare_dram_parameter("src32", [P, T_total], I32,
                                        isOutput=False)
    ind_d = nc.declare_dram_parameter("ind", [P, T_total * P], BF, isOutput=False)
    cnt_d = nc.declare_dram_parameter("cnt", [nblk, P, NVC * P], BF, isOutput=False)
    eoh_d = nc.declare_dram_parameter("eoh", [nblk, P, NVC * P], BF,
                                      isOutput=False)
    ew_d = nc.declare_dram_parameter("ew", [NVC * P, H], BF, isOutput=False)
    cw_d = nc.declare_dram_parameter("cw", [NVC * P, H], BF, isOutput=False)
    gind_d = nc.declare_dram_parameter("gind", [nblk, P, G], FP, isOutput=False)
    ws_d = nc.declare_dram_parameter("W_self", [L, H, H], FP, isOutput=False)
    wn_d = nc.declare_dram_parameter("W_neigh", [L, H, H], FP, isOutput=False)
    wsum_d = nc.declare_dram_parameter("W_sum", [L, H, H], FP, isOutput=False)
    tflag_d = nc.declare_dram_parameter("tflag", [P, 1], FP, isOutput=False)
    bcol_d = nc.declare_dram_parameter("b_cols", [H, L], FP, isOutput=False)
    gcol_d = nc.declare_dram_parameter("gam_cols", [H, L], FP, isOutput=False)
    becol_d = nc.declare_dram_parameter("bet_cols", [H, L], FP, isOutput=False)
    acol_d = nc.declare_dram_parameter("alp_cols", [H, L], FP, isOutput=False)
    out_d = nc.declare_dram_parameter("out", [G, L * H], FP, isOutput=True)

    # z exchanged/gathered in bf16: halves the AllGather on the critical path
    # and the per-edge gather bytes; everything downstream accumulates fp32
    h_shard = nc.dram_tensor("h_shard", [npc, H], BF)
    # double-buffered: the mid-stream half-AllGather of layer l's z must not
    # clobber the table layer l's own remaining gathers still read
    h_fulls = [nc.dram_tensor(f"h_full{i}", [N, H], BF, addr_space="Shared")
               for i in range(2)]
    stats_loc = nc.dram_tensor("stats_loc", [2, H], FP)
    stats_red = nc.dram_tensor("stats_red", [2, H], FP, addr_space="Shared")
    # pooled z held TRANSPOSED [H, G] so the channel affine uses per-partition
    # scalars; transposed back to [G, H] only at the very end
    pool_loc = nc.dram_tensor("pool_loc", [L, H, G], FP)
    pool_red = nc.dram_tensor("pool_red", [L, H, G], FP, addr_space="Shared")

    groups = [list(range(N_CORES))]

    with tile.TileContext(nc) as tc:
        with (
            tc.tile_pool(name="res", bufs=1) as res,
            tc.tile_pool(name="wrk", bufs=3) as wrk,
            tc.tile_pool(name="gat", bufs=3) as gat,
            tc.tile_pool(name="ps_slot", bufs=3, space="PSUM") as ps_slot,
            tc.tile_pool(name="ps_tp", bufs=2, space="PSUM") as ps_tp,
            tc.tile_pool(name="ps_rst", bufs=2, space="PSUM") as ps_rst,
            tc.tile_pool(name="ps_pool", bufs=1, space="PSUM") as ps_pool,
        ):
            ident = res.tile([P, P], FP, tag="ident")
            make_identity(nc, ident[:])

            src32_sb = res.tile([P, T_total], I32, tag="src32")
            nc.sync.dma_start(src32_sb[:], src32_d[:])
            ew_sb = res.tile([P, NVC * H], BF, tag="ew")
            cw_sb = res.tile([P, NVC * H], BF, tag="cw")
            for c in range(NVC):
                nc.sync.dma_start(ew_sb[:, c * H:(c + 1) * H],
                                  ew_d[c * P:(c + 1) * P, :])
                nc.sync.dma_start(cw_sb[:, c * H:(c + 1) * H],
                                  cw_d[c * P:(c + 1) * P, :])
            ws_sb = res.tile([P, L * H], FP, tag="ws")
            wn_sb = res.tile([P, L * H], FP, tag="wn")
            wsum_sb = res.tile([P, L * H], FP, tag="wsum")
            for l in range(L):
                nc.sync.dma_start(ws_sb[:, l * H:(l + 1) * H], ws_d[l])
                nc.sync.dma_start(wn_sb[:, l * H:(l + 1) * H], wn_d[l])
                nc.sync.dma_start(wsum_sb[:, l * H:(l + 1) * H], wsum_d[l])
            tflag_sb = res.tile([P, 1], FP, tag="tflag")
            nc.sync.dma_start(tflag_sb[:], tflag_d[:])
            bcol_sb = res.tile([P, L], FP, tag="bcol")
            nc.sync.dma_start(bcol_sb[:], bcol_d[:])
            gcol_sb = res.tile([P, L], FP, tag="gcol")
            nc.sync.dma_start(gcol_sb[:], gcol_d[:])
            becol_sb = res.tile([P, L], FP, tag="becol")
            nc.sync.dma_start(becol_sb[:], becol_d[:])
            acol_sb = res.tile([P, L], FP, tag="acol")
            nc.sync.dma_start(acol_sb[:], acol_d[:])

            h_stage = res.tile([P, nblk * P], FP, tag="hstage")
            stats_sum = res.tile([P, nblk], FP, tag="ssum")
            stats_sq = res.tile([P, nblk], FP, tag="ssq")
            scratch = res.tile([P, P], FP, tag="scratch")
            eps_col = res.tile([P, 1], FP, tag="eps")
            nc.vector.memset(eps_col[:], float(EPS))
            # per-layer BN affine (s, t), folded weights and bias columns
            s_all = res.tile([P, L], FP, tag="sall")
            t_all = res.tile([P, L], FP, tag="tall")
            wsf_sb = res.tile([P, L * H], FP, tag="wsf")
            wnf_sb = res.tile([P, L * H], FP, tag="wnf")
            biasf_sb = res.tile([P, L], FP, tag="biasf")
            pl_all = res.tile([P, L * H], FP, tag="plall")

            def emit_agg_block(bI, h_full):
                """Gather + indicator-accumulate the 128-node block's
                aggregation directly in PSUM; returns node-row agg tile."""
                nt = tb[bI]
                t0 = toff[bI]
                gt = gat.tile([P, TBMAX * H], BF, tag="g")
                if "gather" not in ablate:
                    for ti in range(nt):
                        nc.gpsimd.indirect_dma_start(
                            out=gt[:, ti * H:(ti + 1) * H],
                            out_offset=None, in_=h_full[:],
                            in_offset=bass.IndirectOffsetOnAxis(
                                ap=src32_sb[:, t0 + ti:t0 + ti + 1],
                                axis=0))
                it = wrk.tile([P, TBMAX * P], BF, tag="indblk")
                nc.sync.dma_start(
                    it[:, :nt * P], ind_d[:, t0 * P:(t0 + nt) * P])
                ps = ps_slot.tile([P, H], FP, tag="slot")
                for ti in range(nt):
                    nc.tensor.matmul(
                        ps[:],
                        lhsT=it[:, ti * P:(ti + 1) * P],
                        rhs=gt[:, ti * H:(ti + 1) * H],
                        start=(ti == 0), stop=(ti == nt - 1))
                ab = wrk.tile([P, H], FP, tag="mablk")
                nc.vector.tensor_copy(ab[:], ps[:])
                return ab

            def emit_stats_post(j):
                """s_j, t_j from the (already AllReduced) stats of z^j; fold
                layer j+1's weights/bias; apply the pool affine for layer j."""
                sxr = wrk.tile([P, 1], FP, tag="sxr")
                nc.sync.dma_start(sxr[:, 0:1], stats_red[0:1, :])
                sqr = wrk.tile([P, 1], FP, tag="sqr")
                nc.sync.dma_start(sqr[:, 0:1], stats_red[1:2, :])
                mu = wrk.tile([P, 1], FP, tag="mu")
                nc.scalar.mul(mu[:], sxr[:], 1.0 / N)
                ex2 = wrk.tile([P, 1], FP, tag="ex2")
                nc.scalar.mul(ex2[:], sqr[:], 1.0 / N)
                mu2 = wrk.tile([P, 1], FP, tag="mu2")
                nc.scalar.square(mu2[:], mu[:])
                var = wrk.tile([P, 1], FP, tag="var")
                nc.vector.tensor_sub(var[:], ex2[:], mu2[:])
                sd = wrk.tile([P, 1], FP, tag="sd")
                nc.scalar.activation(sd[:], var[:],
                                     mybir.ActivationFunctionType.Sqrt,
                                     bias=eps_col[:])
                rstd = wrk.tile([P, 1], FP, tag="rstd")
                nc.vector.reciprocal(rstd[:], sd[:])
                s_col = s_all[:, j:j + 1]
                t_col = t_all[:, j:j + 1]
                nc.vector.tensor_mul(s_col, rstd[:], gcol_sb[:, j:j + 1])
                msc = wrk.tile([P, 1], FP, tag="msc")
                nc.vector.tensor_mul(msc[:], mu[:], s_col)
                nc.vector.tensor_sub(t_col, becol_sb[:, j:j + 1], msc[:])
                if j < L - 1:
                    ln = j + 1
                    nc.vector.tensor_scalar_mul(
                        wsf_sb[:, ln * H:(ln + 1) * H],
                        ws_sb[:, ln * H:(ln + 1) * H], s_col)
                    nc.vector.tensor_scalar_mul(
                        wnf_sb[:, ln * H:(ln + 1) * H],
                        wn_sb[:, ln * H:(ln + 1) * H], s_col)
                    ps_b = ps_rst.tile([P, H], FP, tag="rst")
                    nc.tensor.matmul(ps_b[:, 0:1],
                                     lhsT=wsum_sb[:, ln * H:(ln + 1) * H],
                                     rhs=t_col, start=True, stop=True)
                    nc.vector.tensor_add(biasf_sb[:, ln:ln + 1], ps_b[:, 0:1],
                                         bcol_sb[:, ln:ln + 1])
                # pool affine for layer j: [H, G] = s*poolT + t (core 0 only)
                ps_t = ps_tp.tile([P, P], FP, tag="tp")
                nc.tensor.transpose(out=ps_t[:],
                                    in_=pl_all[:, j * H:(j + 1) * H],
                                    identity=ident[:])
                poolT = wrk.tile([P, P], FP, tag="poolT")
                nc.scalar.copy(poolT[:], ps_t[:])
                tf = wrk.tile([P, 1], FP, tag="tf")
                nc.vector.tensor_mul(tf[:], t_col, tflag_sb[:])
                pla = wrk.tile([P, G], FP, tag="pla")
                nc.vector.scalar_tensor_tensor(
                    pla[:], poolT[:, :G], s_col, tf[:].to_broadcast([P, G]),
                    op0=mybir.AluOpType.mult, op1=mybir.AluOpType.add)
                nc.sync.dma_start(pool_loc[j], pla[:])

            def emit_block(l, bI, ps_p):
                    nn = last if bI == nblk - 1 else P
                    ps_r = ps_rst.tile([P, H], FP, tag="rst")
                    if l == 0:
                        # rst_pre0^T = (emb@Ws)^T EohT + (emb@Wn)^T cntT,
                        # all host-folded: 6 accumulating matmuls, no gathers
                        eoh_sb = wrk.tile([P, NVC * P], BF, tag="eohblk")
                        nc.sync.dma_start(eoh_sb[:], eoh_d[bI])
                        cnt_sb = wrk.tile([P, NVC * P], BF, tag="cntblk")
                        nc.sync.dma_start(cnt_sb[:], cnt_d[bI])
                        for cv in range(NVC):
                            nc.tensor.matmul(
                                ps_r[:],
                                lhsT=ew_sb[:, cv * H:(cv + 1) * H],
                                rhs=eoh_sb[:, cv * P:(cv + 1) * P],
                                start=(cv == 0), stop=False)
                        for cv in range(NVC):
                            nc.tensor.matmul(
                                ps_r[:],
                                lhsT=cw_sb[:, cv * H:(cv + 1) * H],
                                rhs=cnt_sb[:, cv * P:(cv + 1) * P],
                                start=False, stop=(cv == NVC - 1))
                        bc = bcol_sb[:, 0:1]
                    else:
                        ab = emit_agg_block(bI, h_fulls[(l - 1) % 2])
                        ps_t = ps_tp.tile([P, P], FP, tag="tp")
                        nc.tensor.transpose(out=ps_t[:], in_=ab[:],
                                            identity=ident[:])
                        aT = wrk.tile([P, P], FP, tag="aT")
                        nc.scalar.copy(aT[:], ps_t[:])
                        rhs_self = h_stage[:, bI * P:(bI + 1) * P]
                        bc = biasf_sb[:, l:l + 1]
                        nc.tensor.matmul(ps_r[:],
                                         lhsT=wsf_sb[:, l * H:(l + 1) * H],
                                         rhs=rhs_self, start=True, stop=False)
                        nc.tensor.matmul(ps_r[:],
                                         lhsT=wnf_sb[:, l * H:(l + 1) * H],
                                         rhs=aT[:], start=False, stop=True)

                    t1 = wrk.tile([P, P], FP, tag="t1")
                    nc.scalar.activation(t1[:], ps_r[:],
                                         mybir.ActivationFunctionType.Relu,
                                         bias=bc)
                    neg = wrk.tile([P, P], FP, tag="neg")
                    nc.vector.tensor_scalar(
                        neg[:], ps_r[:], bc, 0.0,
                        op0=mybir.AluOpType.add, op1=mybir.AluOpType.min)
                    zb = h_stage[:, bI * P:(bI + 1) * P]
                    if nn == P:
                        nc.vector.scalar_tensor_tensor(
                            zb, neg[:], acol_sb[:, l:l + 1], t1[:],
                            op0=mybir.AluOpType.mult, op1=mybir.AluOpType.add,
                            accum_out=stats_sum[:, bI:bI + 1])
                        nc.scalar.activation(scratch[:], zb,
                                             mybir.ActivationFunctionType.Square,
                                             accum_out=stats_sq[:, bI:bI + 1])
                    else:
                        nc.vector.scalar_tensor_tensor(
                            h_stage[:, bI * P:bI * P + nn],
                            neg[:, :nn], acol_sb[:, l:l + 1], t1[:, :nn],
                            op0=mybir.AluOpType.mult, op1=mybir.AluOpType.add,
                            accum_out=stats_sum[:, bI:bI + 1])
                        nc.vector.scalar_tensor_tensor(
                            h_stage[:, bI * P + nn:(bI + 1) * P],
                            neg[:, nn:], acol_sb[:, l:l + 1], t1[:, nn:],
                            op0=mybir.AluOpType.mult, op1=mybir.AluOpType.add)
                        nc.scalar.activation(
                            scratch[:, :nn], h_stage[:, bI * P:bI * P + nn],
                            mybir.ActivationFunctionType.Square,
                            accum_out=stats_sq[:, bI:bI + 1])

                    # fused tail (old pass B): transpose z to node rows,
                    # write the shard, accumulate the raw-z pool
                    ps_t2 = ps_tp.tile([P, P], FP, tag="tp")
                    nc.tensor.transpose(out=ps_t2[:], in_=zb,
                                        identity=ident[:])
                    hnm = wrk.tile([P, P], FP, tag="hnm")
                    nc.scalar.copy(hnm[:], ps_t2[:])
                    if l < L - 1:
                        hnm_bf = wrk.tile([P, P], BF, tag="hnmbf")
                        nc.scalar.copy(hnm_bf[:], ps_t2[:])
                        nc.sync.dma_start(
                            h_shard[bI * P:bI * P + nn, :], hnm_bf[:nn, :])
                    gb = wrk.tile([P, G], FP, tag="gblk")
                    nc.sync.dma_start(gb[:], gind_d[bI])
                    nc.tensor.matmul(ps_p[:G, :], lhsT=gb[:], rhs=hnm[:],
                                     start=(bI == 0), stop=(bI == nblk - 1))

            def emit_layer_tail(l, ps_p):
                nc.vector.tensor_copy(pl_all[:G, l * H:(l + 1) * H],
                                      ps_p[:G, :])
                # per-channel z sums for this layer's BN stats
                sx = wrk.tile([P, 1], FP, tag="sx")
                nc.vector.tensor_reduce(sx[:], stats_sum[:],
                                        axis=mybir.AxisListType.X,
                                        op=mybir.AluOpType.add)
                sq = wrk.tile([P, 1], FP, tag="sq")
                nc.vector.tensor_reduce(sq[:], stats_sq[:],
                                        axis=mybir.AxisListType.X,
                                        op=mybir.AluOpType.add)
                nc.sync.dma_start(stats_loc[0:1, :], sx[:, 0:1])
                nc.sync.dma_start(stats_loc[1:2, :], sq[:, 0:1])

            # ---------------- schedule ----------------
            HS = npc // 2      # covered by blocks [0, HS//P]; trigger later
            SPLIT_BLK = 64     # with margin so the pool queue never stalls

            def emit_ag_half(l, first):
                h_full = h_fulls[l % 2]
                if first:
                    nc.gpsimd.collective_compute(
                        "AllGather", mybir.AluOpType.bypass,
                        replica_groups=groups,
                        ins=[h_shard[0:HS]], outs=[h_full[0:N // 2]])
                else:
                    nc.gpsimd.collective_compute(
                        "AllGather", mybir.AluOpType.bypass,
                        replica_groups=groups,
                        ins=[h_shard[HS:]], outs=[h_full[N // 2:]])

            for l in range(L):
                ps_p = ps_pool.tile([P, H], FP, tag="pool")
                if l > 0:
                    # the previous layer's stats postprocessing overlaps this
                    # layer's gather stream (its deps resolve early; only the
                    # first W-matmuls wait on the folded weights)
                    emit_stats_post(l - 1)
                for bI in range(nblk):
                    emit_block(l, bI, ps_p)
                    if l < L - 1 and bI == SPLIT_BLK:
                        # first half-shard exchange fires mid-stream
                        emit_ag_half(l, True)
                emit_layer_tail(l, ps_p)
                if l < L - 1:
                    emit_ag_half(l, False)
                nc.gpsimd.collective_compute(
                    "AllReduce", mybir.AluOpType.add, replica_groups=groups,
                    ins=[stats_loc[:]], outs=[stats_red[:]])
            emit_stats_post(L - 1)

            nc.gpsimd.collective_compute(
                "AllReduce", mybir.AluOpType.add, replica_groups=groups,
                ins=[pool_loc[:]], outs=[pool_red[:]])
            for l in range(L):
                pr = wrk.tile([P, G], FP, tag="pr")
                nc.sync.dma_start(pr[:], pool_red[l])
                ps_o = ps_tp.tile([P, P], FP, tag="tp")
                nc.tensor.transpose(out=ps_o[:], in_=pr[:], identity=ident[:])
                ob = wrk.tile([P, H], FP, tag="ob")
                nc.scalar.copy(ob[:], ps_o[:])
                nc.sync.dma_start(out_d[:, l * H:(l + 1) * H], ob[:G, :])

    nc.compile()
    return nc


# ---------------------------------------------------------------------------
# entry point
# ---------------------------------------------------------------------------

_CACHE = {}


def _run(cfg, inputs, trace=False):
    from concourse.bass_utils import run_bass_kernel_spmd
    in_maps, tb = _prep(cfg, **inputs)
    key = (cfg["N"], cfg["G"], cfg["H"], tb)
    if key not in _CACHE:
        _CACHE[key] = build_program(cfg, tb)
    nc = _CACHE[key]
    last_exc = None
    for attempt in range(3):
        try:
            return run_bass_kernel_spmd(nc, in_maps, list(range(N_CORES)),
                                        trace=trace)
        except Exception as e:  # rare transient device-unrecoverable errors
            last_exc = e
            try:
                import jax
                import jax.extend.backend
                jax.clear_caches()
                jax.extend.backend.clear_backends()
            except Exception:
                pass
    raise last_exc


def kernel(in_feat, src, dst, graph_ids, emb, W_self, W_neigh, b,
           gamma, beta, prelu_w):
    cfg = _mkcfg(**CFG_FULL)
    res = _run(cfg, dict(
        in_feat=in_feat, src=src, dst=dst, graph_ids=graph_ids, emb=emb,
        W_self=W_self, W_neigh=W_neigh, b=b, gamma=gamma, beta=beta,
        prelu_w=prelu_w))
    return np.asarray(res.results[0]["out"], np.float32)

